# revision 14
# baseline (speedup 1.0000x reference)
"""Trainium2 Bass kernel for InvariantMessagePassingTP.

out[n, lm, c] = sum_{e: recv[e]=n} edge_attrs[e,lm] * tp_weights[e,l(lm),c]
                * node_feats[recv[e], c]

Since recv[e] = n inside a segment, node_feats factors out of the sum:
  out[n, lm, c] = node_feats[n, c] * sum_{e: recv[e]=n}
                  edge_attrs[e,lm] * tp_weights[e,l(lm),c]
so the device only computes the A*W segment sum; the (cheap, per-node)
node_feats multiply happens on the host during unpack, in f32.

Strategy (8 NeuronCores, SPMD, no collectives):
  receiver_list is sorted -> each core owns a contiguous node range (3125
  nodes) and its contiguous edge range. The host cuts the edge stream into
  dense 128-edge tiles (cut early only when a tile would span >8 distinct
  nodes; a node may span tiles - the host sums both parts). Edges sit on
  SBUF partitions.

  Per tile (the A-fold trick - both A and the one-hot scatter live in the
  matmul stationary):
    At[e, lm*8+k]     = A[e,lm]  * S8[e,k]         (DVE TT, batched over
                        the whole 8-tile PSUM batch;
                        S8 = one-hot of the node's local index k in 0..7)
    P = At^T @ W      (PE, 2 matmuls, fp32 PSUM: P[lm*8+k, l*64+c]
                       = sum_e A*S8*W -- rows (lm,k), col block l(lm)
                       holds the answer; W streams straight from the
                       input chunk, no pre-multiply needed)
  8 tiles share one PSUM tile; ACT then copies each l-column-block of PSUM
  (all 128 lanes) to bf16 staging, and per-l DMAs ship only the valid row
  ranges to DRAM laid out as slots[lm, k, tile, c]. Input loads ride the
  SP HWDGE queue, output stores the ACT HWDGE queue, so prefetch never
  queues behind drains. The host gathers slots -> out[node, lm, c]
  (summing where a node spans tiles), then scales by node_feats.
"""

import sys

sys.path.insert(0, "/opt/trn_rl_repo")

import numpy as np
import ml_dtypes

import concourse.bass as bass
import concourse.bacc as bacc
import concourse.tile as tile
from concourse import mybir
from concourse.bass_utils import run_bass_kernel_spmd

NPBF = ml_dtypes.bfloat16
BF16 = mybir.dt.bfloat16
F32 = mybir.dt.float32

NNODES = 25000
NEDGES = 400000
NCHAN = 64
N_CORES = 8
NPC = NNODES // N_CORES        # nodes per core
TB = 280                       # bf16 elems per tile per partition
CHUNK = 64                     # tiles per input DMA chunk
PSB = 8                        # tiles per PSUM batch

L_OF_LM = np.array([0, 1, 1, 1, 2, 2, 2, 2, 2, 3, 3, 3, 3, 3, 3, 3], np.int64)
# row-block order of lm in At / PSUM / slots: l2,l3 first (96 rows at psum
# base 0), then l0,l1 (base 96) - matmul psum-base constraint.
PERM_LM = [4, 5, 6, 7, 8, 9, 10, 11, 12, 13, 14, 15, 0, 1, 2, 3]

_PROGRAM_CACHE = {}


def _build_schedule(receiver_list):
    """Cut each core's (sorted) edge stream into dense <=128-edge tiles,
    cutting early only when a tile would cover >8 distinct nodes.
    Returns per-core tile lists [(e0, e1, node_ids)] (global edge idx)."""
    recv = np.asarray(receiver_list).astype(np.int64)
    node_e0 = np.searchsorted(recv, np.arange(NNODES + 1))
    per_core = []
    for c in range(N_CORES):
        e_lo, e_hi = node_e0[c * NPC], node_e0[(c + 1) * NPC]
        tiles = []
        i = e_lo
        while i < e_hi:
            j = min(i + 128, e_hi)
            w = recv[i:j]
            uniq = np.unique(w)
            if len(uniq) > 8:
                j = i + np.searchsorted(w, uniq[8])
                uniq = uniq[:8]
            tiles.append((i, j, uniq))
            i = j
        per_core.append(tiles)
    t_max = max(len(t) for t in per_core)
    t_u = -(-t_max // PSB) * PSB  # round up to PSUM batch
    return recv, per_core, t_u


def _pack_inputs(edge_attrs, tp_weights, recv, per_core, t_u):
    w_bf = np.asarray(tp_weights, np.float32).reshape(NEDGES, 256).astype(NPBF)
    a_bf = np.asarray(edge_attrs, np.float32).astype(NPBF)

    in_maps = []
    slot_maps = []  # per core: list of node_id arrays per tile
    for c in range(N_CORES):
        tiles = per_core[c]
        T = t_u
        # slot-major staging [T*128, TB]: [ W 0:256 | A 256:272 | S8 272:280 ]
        X = np.zeros((T * 128, TB), NPBF)
        smap = []
        for t, (e0, e1, uniq) in enumerate(tiles):
            ne = e1 - e0
            base = t * 128
            X[base:base + ne, 0:256] = w_bf[e0:e1]
            X[base:base + ne, 256:272] = a_bf[e0:e1][:, PERM_LM]
            loc = np.searchsorted(uniq, recv[e0:e1])  # 0..7
            X[base + np.arange(ne), 272 + loc] = NPBF(1.0)
            smap.append(uniq)
        while len(smap) < T:
            smap.append(np.empty(0, np.int64))
        # chunk-block-major device layout
        Xt = X.reshape(T, 128, TB)
        n_chunks = -(-T // CHUNK)
        buf = np.zeros((128, T * TB), NPBF)
        pos = 0
        for ch in range(n_chunks):
            t0, t1 = ch * CHUNK, min((ch + 1) * CHUNK, T)
            for so, sz in ((0, 256), (256, 16), (272, 8)):
                blk = Xt[t0:t1, :, so:so + sz]  # [ct, 128, sz]
                ct = t1 - t0
                buf[:, pos:pos + ct * sz] = (
                    blk.transpose(1, 0, 2).reshape(128, ct * sz))
                pos += ct * sz
        in_maps.append({"inp": buf})
        slot_maps.append(smap)
    return in_maps, slot_maps


def _build_program(t_u):
    nc = bacc.Bacc("TRN2", target_bir_lowering=False, debug=False,
                   num_devices=N_CORES)
    T = t_u
    in_d = nc.dram_tensor("inp", [128, T * TB], BF16, kind="ExternalInput").ap()
    # slots[row = perm-lm-block*8 + k, tile, c]
    out_d = nc.dram_tensor("out", [128, T, 64], BF16,
                           kind="ExternalOutput").ap()

    n_chunks = -(-T // CHUNK)
    with tile.TileContext(nc) as tc:
        with tc.tile_pool(name="ld", bufs=3) as ld_pool, \
             tc.tile_pool(name="at", bufs=8) as at_pool, \
             tc.tile_pool(name="st", bufs=2) as st_pool, \
             tc.tile_pool(name="ps", bufs=4, space="PSUM") as ps_pool:
            for ch in range(n_chunks):
                t0, t1 = ch * CHUNK, min((ch + 1) * CHUNK, T)
                ct = t1 - t0
                # chunk block offsets (bf16 elems within the chunk)
                oW, oA, oS = 0, ct * 256, ct * 272
                base_el = t0 * TB
                ld = ld_pool.tile([128, ct * TB], BF16, tag="ld")
                nc.sync.dma_start(
                    out=ld,
                    in_=bass.AP(
                        tensor=in_d.tensor, offset=base_el,
                        ap=[[T * TB, 128], [1, ct * TB]]),
                )
                # per-chunk staging: [128, half, ct, 64] bf16
                stage = st_pool.tile([128, 2, ct, 64], BF16, tag="stage")
                for p0 in range(0, ct, PSB):
                    ps = ps_pool.tile([128, PSB, 128], F32, tag="ps")
                    # At[e, t, lm*8 + k] = A[e,t,lm] * S8[e,t,k]
                    # one DVE op for the whole PSUM batch
                    at8 = at_pool.tile([128, PSB, 128], BF16, tag="at")
                    a_v = ld[:, oA + p0 * 16: oA + (p0 + PSB) * 16]
                    s_v = ld[:, oS + p0 * 8: oS + (p0 + PSB) * 8]
                    nc.vector.tensor_mul(
                        at8.rearrange("p t (l k) -> p t l k", l=16),
                        a_v.rearrange("p (t l) -> p t l", t=PSB)[
                            :, :, :, None].broadcast_to(
                                [128, PSB, 16, 8]),
                        s_v.rearrange("p (t k) -> p t k", t=PSB)[
                            :, :, None, :].broadcast_to(
                                [128, PSB, 16, 8]),
                    )
                    # phase A: rows 0-95 = (l2|l3) x W cols 128:256
                    for k in range(PSB):
                        w0 = oW + (p0 + k) * 256
                        nc.tensor.matmul(
                            ps[0:96, k], at8[:, k, 0:96],
                            ld[:, w0 + 128: w0 + 256],
                            start=True, stop=True)
                    # phase B: rows 96-127 = (l0|l1) x W cols 0:128
                    for k in range(PSB):
                        w0 = oW + (p0 + k) * 256
                        nc.tensor.matmul(
                            ps[96:128, k], at8[:, k, 96:128],
                            ld[:, w0: w0 + 128],
                            start=True, stop=True,
                            tile_position=(0, 96))
                    # full-lane extraction of the whole PSUM batch into the
                    # chunk stage, col halves separated for contiguous DMA
                    nc.scalar.copy(
                        bass.AP(
                            tensor=stage.tensor, offset=stage.offset + p0 * 64,
                            ap=[stage.ap[0], [64, PSB], [ct * 64, 2],
                                [1, 64]]),
                        ps,
                    )
                # 4 out-DMA fragments per chunk on the ACT HWDGE queue;
                # DMA picks valid rows
                for (r0, r1, half) in ((0, 40, 0), (40, 96, 1),
                                       (96, 104, 0), (104, 128, 1)):
                    nc.scalar.dma_start(
                        out=bass.AP(
                            tensor=out_d.tensor,
                            offset=r0 * (T * 64) + t0 * 64,
                            ap=[[T * 64, r1 - r0], [64, ct], [1, 64]]),
                        in_=stage[r0:r1, half],
                    )
    nc.compile()
    return nc


def kernel(node_feats, edge_attrs, tp_weights, receiver_list, nnodes,
           _trace=False):
    node_feats = np.asarray(node_feats)
    edge_attrs = np.asarray(edge_attrs)
    tp_weights = np.asarray(tp_weights)
    receiver_list = np.asarray(receiver_list)
    nnodes = int(nnodes)
    assert node_feats.shape == (NNODES, NCHAN) and nnodes == NNODES
    assert tp_weights.shape == (NEDGES, 4, NCHAN)

    recv, per_core, t_u = _build_schedule(receiver_list)
    key = int(t_u)
    if key not in _PROGRAM_CACHE:
        _PROGRAM_CACHE[key] = _build_program(t_u)
    nc = _PROGRAM_CACHE[key]

    in_maps, slot_maps = _pack_inputs(
        edge_attrs, tp_weights, recv, per_core, t_u)
    res = run_bass_kernel_spmd(nc, in_maps, list(range(N_CORES)),
                               trace=_trace)

    inv = np.argsort(np.array(PERM_LM))  # lm -> row-block index
    out = np.zeros((NNODES, 16, NCHAN), np.float32)
    for c in range(N_CORES):
        slots = res.results[c]["out"].astype(np.float32)  # [128, T, 64]
        slots = slots.reshape(16, 8, -1, NCHAN)[inv]  # [lm, k, T, c]
        smap = slot_maps[c]
        for t, uniq in enumerate(smap):
            k = len(uniq)
            if k == 0:
                continue
            out[uniq] += slots[:, 0:k, t, :].transpose(1, 0, 2)
    # node_feats factors out of the segment sum; apply in f32 on the host
    out *= np.asarray(node_feats, np.float32)[:, None, :]
    if _trace:
        return out, res
    return out
